# revision 33
# baseline (speedup 1.0000x reference)
"""Trainium2 Bass kernel for the CSA (channel-spatial attention) module.

Reference computation (per batch b):
    q = Wq @ x[b]            # [64, N]
    k = Wk @ x[b]            # [64, N]
    E[n, m] = sum_c q[c, n] * k[c, m]          # [N, N]
    A = softmax(E, axis=m)
    v = Wv @ x_h[b]          # [128, N]
    out[c, n] = sum_m v[c, m] * A[n, m]
    result = gamma * out + x_h[b]

Sharding: 8 cores = 4 batches x 2 query-halves. Each core holds full K/V for
its batch and a 2048-wide query chunk (flash-style: the [N, N] attention
matrix is never materialized in HBM).

The kernel is ACT(exp)-roofline bound: 8.39M exps/core at 1 elem/cycle/lane
@1.2GHz, with a (N+352)cyc/instruction cost. PSUM's 8 banks set the ACT
instruction width; this version squeezes the bank budget to afford
double-buffered THREE-bank E-trios ([128, 1536] f32), cutting the stream
from 64x(1024+352) to 44 instructions (~67.5us steady):
  e trios 2x3 banks + U 1 bank + S/scratch 1 bank = 8.
- U fits one bank by copying each group's finished U to SBUF (DVE) in the
  ACT shadow before the next group's accumulation overwrites the bank.
- The vT-projection scratch tiles time-share the S bank: all of vT is
  built during group 0 while group 0's S matmuls are buffered on DVE and
  flushed after the last vT block releases the bank.
- Host folds the tiny 1x1-conv projections: qk = (Wk^T Wq) @ x_half and
  gamma into Wv^T. The attention math (E/softmax/U) runs on device.
- Input DMA is bandwidth+latency limited (~2.3us engine startup): only
  2.3MB/core is loaded (no zero-pad rows, bf16 residual); E's stationary
  and moving pad rows are zero-filled on-chip via DVE copies.
- PE warm-up matmuls at t=0 (into the first E-trio's slices) keep the HAM
  clock gate from starting the stream cold; a 1-element dummy exp pulls
  the ~2.7us ACT_TABLE_LOAD off the critical path.
- out = U/S + x_h via reciprocal_approx_fast + gpsimd partition_broadcast;
  the last group's epilogue pipelines in column halves.
- No max-subtraction: logits are N(0, 64); |E| < ~65 << 88 (exp overflow).
"""

import numpy as np

import concourse.bass as bass
import concourse.mybir as mybir
import concourse.tile as tile
from concourse import bacc
from concourse.bass_utils import run_bass_kernel_spmd

B = 4
CQK = 64
CV = 128
N = 4096
NQ = N // 2          # query columns per core
NG = 512             # n-group width (PSUM bank / U matmul free dim)
MT = 128             # m-tile height (PE contraction tile)
TW = 3 * NG          # E-trio width: 3 m-tiles side by side (3 PSUM banks)
N_GROUPS = NQ // NG  # 4
N_TILES_G = 11       # 10 trios + 1 final pair = 32 m-tiles per group
NPT = N_GROUPS * N_TILES_G
N_WARM = 6           # PE warm-up matmuls (fill the DMA wait, prime HAM)
PIPE = 2             # E-tile pipeline depth

F32 = mybir.dt.float32
BF16 = mybir.dt.bfloat16


_last_results = None  # stashed BassKernelResults for test harnesses


def _mtiles(t):
    """m-tile indices covered by tile t of a group (trios, then a pair)."""
    return range(3 * t, 3 * t + 3) if t < 10 else range(30, 32)


def build_bass() -> bass.Bass:
    nc = bacc.Bacc()

    xb = nc.declare_dram_parameter("xb", [CQK, N], BF16, isOutput=False)
    xhb = nc.declare_dram_parameter("xhb", [CV, N], BF16, isOutput=False)
    qkh = nc.declare_dram_parameter("qkh", [CQK, NQ], BF16, isOutput=False)
    xh_res = nc.declare_dram_parameter("xh_res", [CV, NQ], BF16, isOutput=False)
    wvT = nc.declare_dram_parameter("wvT", [CV, CV], BF16, isOutput=False)
    o = nc.declare_dram_parameter("o", [CV, NQ], F32, isOutput=True)

    ts = bass.ts

    with tile.TileContext(nc) as tc:
        with (
            nc.allow_low_precision(reason="bf16 attention math, fp32 accum"),
            tc.tile_pool(name="const", bufs=1) as cpool,
            tc.tile_pool(name="pt", bufs=3) as ptpool,
            tc.tile_pool(name="red", bufs=2) as rpool,
            tc.tile_pool(name="sacc", bufs=6) as saccpool,
            tc.tile_pool(name="ep", bufs=PIPE, space="PSUM") as epool,
            tc.tile_pool(name="up", bufs=1, space="PSUM") as upool,
            tc.tile_pool(name="sp", bufs=1, space="PSUM") as spool,
            tc.tile_pool(name="out", bufs=3) as opool,
        ):
            # ---- persistent SBUF tensors ----
            xb_sb = cpool.tile([MT, N], BF16)    # rows CQK.. zero-filled on-chip
            xhb_sb = cpool.tile([CV, N], BF16)
            xhres_sb = cpool.tile([CV, NQ], BF16)
            wvT_sb = cpool.tile([CV, CV], BF16)
            qk_sb = cpool.tile([MT, NQ], BF16)   # rows CQK.. zero-filled on-chip
            vT_sb = cpool.tile([CV, N], BF16)    # cols [mt*128,..) = v[:, chunk].T
            zwarm = cpool.tile([MT, NG], BF16)   # zeros: warm-up + pad source
            ones_m = cpool.tile([MT, 1], BF16)   # S-matmul stationary
            ones_stage = cpool.tile([MT, 1], F32)

            # ---- t=0: warm-up + table preload + DMA prologue ----
            nc.gpsimd.memset(zwarm[:], 0.0)

            # preload the exp table set while the DMAs run
            tl_sb = opool.tile([MT, 1], F32, tag="o", name="tl")
            nc.scalar.activation(tl_sb[:], zwarm[:, :1],
                                 mybir.ActivationFunctionType.Exp, bias=0.0)

            # first E-trio tile doubles as the warm-up target: the real E
            # matmuls overwrite it in-order on the PE, so this costs nothing
            e_first = epool.tile([MT, TW], F32, tag="e", name="e_0_0")
            for w in range(N_WARM):
                nc.tensor.matmul(e_first[:, ts(w % 3, NG)], zwarm[:, :MT],
                                 zwarm[:], start=True, stop=True)

            # sync queue: loop-critical tensors first, in consumption order;
            # leading chunks are small so E can start ~4us before the bulk.
            nc.sync.dma_start(qk_sb[:CQK, :NG], qkh[:, :NG])
            nc.sync.dma_start(xb_sb[:CQK, ts(0, NG)], xb[:, ts(0, NG)])
            nc.sync.dma_start(wvT_sb[:], wvT[:])
            nc.sync.dma_start(xb_sb[:CQK, ts(1, NG)], xb[:, ts(1, NG)])
            nc.sync.dma_start(qk_sb[:CQK, NG:], qkh[:, NG:])
            for j in range(2, N // NG):
                nc.sync.dma_start(xb_sb[:CQK, ts(j, NG)], xb[:, ts(j, NG)])
            nc.sync.dma_start(xhres_sb[:], xh_res[:])
            # gpsimd queue: vT inputs, one chunk per vblk
            for j in range(N // NG):
                nc.gpsimd.dma_start(xhb_sb[:, ts(j, NG)], xhb[:, ts(j, NG)])
            nc.gpsimd.memset(ones_stage[:], 1.0)

            # on-chip zero padding (DVE; zwarm is the zero source)
            for j in range(NQ // NG):
                nc.vector.tensor_copy(qk_sb[CQK:, ts(j, NG)], zwarm[:CQK, :])
            for j in range(2):
                nc.vector.tensor_copy(xb_sb[CQK:, ts(j, 2048)], qk_sb[CQK:, :])
            nc.vector.tensor_copy(ones_m[:], ones_stage[:])

            # ---- vT projection block j (time-shares the S bank) ----
            def emit_vblk(j):
                vt_ps = spool.tile([CV, NG], F32, tag="s", name=f"vtp_{j}")
                for u in range(4):
                    mt = j * 4 + u
                    nc.tensor.matmul(vt_ps[:, ts(u, MT)], xhb_sb[:, ts(mt, MT)],
                                     wvT_sb[:], start=True, stop=True)
                nc.vector.tensor_copy(vT_sb[:, ts(j, NG)], vt_ps[:])

            # ---- E-tile: trio (or final pair) of m-tiles, f32 PSUM ----
            def emit_Etile(g, t, tile_=None):
                mts = list(_mtiles(t))
                e = tile_ if tile_ is not None else epool.tile(
                    [MT, NG * len(mts)], F32, tag="e", name=f"e_{g}_{t}")
                for u, mt in enumerate(mts):
                    nc.tensor.matmul(e[:, ts(u, NG)], xb_sb[:, ts(mt, MT)],
                                     qk_sb[:, ts(g, NG)], start=True, stop=True)
                return e

            def emit_epilogue(g, u_src, s_ps, split=1):
                # out = U / S + x_h   (gamma pre-folded into wvT on the host)
                w = NG // split
                for h in range(split):
                    sl = slice(h * w, (h + 1) * w)
                    r_sb = opool.tile([1, w], F32, tag="r", name=f"r_{g}_{h}")
                    nc.vector.reciprocal_approx_fast(out=r_sb[:],
                                                     in_=s_ps[:1, sl])
                    rb_sb = opool.tile([CV, w], F32, tag="rb",
                                       name=f"rb_{g}_{h}")
                    nc.gpsimd.partition_broadcast(rb_sb[:], r_sb[:])
                    o_sb = opool.tile([CV, w], F32, tag="o", name=f"o_{g}_{h}")
                    nc.vector.tensor_mul(o_sb[:], u_src[:, sl], rb_sb[:])
                    nc.vector.tensor_add(o_sb[:], o_sb[:],
                                         xhres_sb[:, g * NG + h * w:
                                                   g * NG + (h + 1) * w])
                    nc.sync.dma_start(o[:, g * NG + h * w:
                                        g * NG + (h + 1) * w], o_sb[:])

            def emit_S(s_ps, first, last, ptt):
                nc.tensor.matmul(s_ps[:1, :], ones_m[:], ptt[:],
                                 start=first, stop=last)

            # ---- main flash loop over 44 E-tiles, software-pipelined ----
            e_tiles = {0: emit_Etile(0, 0, tile_=e_first),
                       1: emit_Etile(0, 1)}
            for j in range(2):
                emit_vblk(j)
            u_ps = s_ps = None
            s_first = True
            pending = None
            pending_s = []
            raw_prev = None
            for p in range(NPT):
                g, t = divmod(p, N_TILES_G)
                mts = list(_mtiles(t))
                tw = NG * len(mts)
                if t == 0:
                    u_ps = upool.tile([CV, NG], F32, tag="u", name=f"u_{g}")
                pt = ptpool.tile([MT, TW], BF16, tag="pt", name=f"pt_{g}_{t}")
                nc.scalar.activation(pt[:, :tw], e_tiles.pop(p)[:],
                                     mybir.ActivationFunctionType.Exp,
                                     bias=0.0)
                if p + PIPE < NPT:
                    gn, tn = divmod(p + PIPE, N_TILES_G)
                    e_tiles[p + PIPE] = emit_Etile(gn, tn)
                # U[c, n] += vT_tile.T @ P^T  (all m-tiles of this E-tile)
                for u, mt in enumerate(mts):
                    nc.tensor.matmul(u_ps[:], vT_sb[:, ts(mt, MT)],
                                     pt[:, ts(u, NG)],
                                     start=(t == 0 and u == 0),
                                     stop=(mt == 31))
                if g == 0 and t < 6:
                    emit_vblk(t + 2)  # vblk k needed by U from trio ~(4k-2)/3
                # softmax-denominator partial for this tile (DVE)
                ptt = saccpool.tile([MT, NG], BF16, tag="ptt",
                                    name=f"pq_{g}_{t}")
                if t < 10:
                    s1 = rpool.tile([MT, NG], BF16, tag="s1", name=f"s1_{g}_{t}")
                    nc.vector.tensor_add(s1[:], pt[:, :NG], pt[:, NG:2 * NG])
                    nc.vector.tensor_add(ptt[:], s1[:], pt[:, 2 * NG:3 * NG])
                else:
                    nc.vector.tensor_add(ptt[:], pt[:, :NG], pt[:, NG:2 * NG])
                lastt = t == N_TILES_G - 1
                # Group 0 buffers S partials (folded pairwise on DVE) until
                # the last vT block releases the S bank; later groups buffer
                # only past the previous group's reciprocal.
                flush_at = 7 if g == 0 else 2
                if t < flush_at:
                    if raw_prev is None:
                        raw_prev = ptt
                    else:
                        f = saccpool.tile([MT, NG], BF16, tag="ptt",
                                          name=f"pf_{g}_{t}")
                        nc.vector.tensor_add(f[:], raw_prev[:], ptt[:])
                        pending_s.append(f)
                        raw_prev = None
                else:
                    if s_ps is None:
                        s_ps = spool.tile([1, NG], F32, tag="s", name=f"s_{g}")
                    if raw_prev is not None:
                        pending_s.append(raw_prev)
                        raw_prev = None
                    for buf in pending_s:
                        emit_S(s_ps, s_first, False, buf)
                        s_first = False
                    pending_s = []
                    emit_S(s_ps, s_first, lastt, ptt)
                    s_first = False
                if pending is not None and (t >= 1 or p == NPT - 1):
                    emit_epilogue(*pending)
                    pending = None
                if lastt:
                    if g < N_GROUPS - 1:
                        # free the single U bank for the next group: copy the
                        # finished U to SBUF before the next accumulation
                        u_sb = opool.tile([CV, NG], F32, tag="ucp",
                                          name=f"ucp_{g}")
                        nc.vector.tensor_copy(u_sb[:], u_ps[:])
                        pending = (g, u_sb, s_ps)
                    else:
                        pending = (g, u_ps, s_ps)
                    s_ps = None
                    s_first = True
            emit_epilogue(*pending, split=2)

    nc.compile()
    return nc


def kernel(x, x_h, Wq, Wk, Wv, gamma):
    global _last_results
    import ml_dtypes
    bf16 = ml_dtypes.bfloat16

    x = np.ascontiguousarray(np.asarray(x, dtype=np.float32))
    x_h = np.ascontiguousarray(np.asarray(x_h, dtype=np.float32))
    Wq = np.asarray(Wq, dtype=np.float32)
    Wk = np.asarray(Wk, dtype=np.float32)
    Wv = np.asarray(Wv, dtype=np.float32)
    gval = float(np.asarray(gamma).reshape(-1)[0])

    nc = build_bass()

    # Fold the 1x1-conv projections on the host:
    #   qk = (Wk^T Wq) @ x_half  (query-key product, bf16)
    #   wvT = gamma * Wv^T       (value projection weights with gamma baked in)
    A = Wk.T @ Wq
    wvT_h = np.ascontiguousarray(Wv.T * gval).astype(bf16)
    x_bf = x.astype(bf16)
    xh_bf = x_h.astype(bf16)

    in_maps = []
    for core in range(8):
        b, h = core // 2, core % 2
        sl = slice(h * NQ, (h + 1) * NQ)
        in_maps.append({
            "xb": x_bf[b],
            "xhb": xh_bf[b],
            "qkh": np.ascontiguousarray((A @ x[b][:, sl]).astype(bf16)),
            "xh_res": np.ascontiguousarray(xh_bf[b][:, sl]),
            "wvT": wvT_h,
        })

    res = run_bass_kernel_spmd(nc, in_maps, list(range(8)))
    _last_results = res

    out = np.empty((B, CV, N), dtype=np.float32)
    for core in range(8):
        b, h = core // 2, core % 2
        out[b][:, h * NQ:(h + 1) * NQ] = res.results[core]["o"]
    return out


# revision 34
# speedup vs baseline: 1.1898x; 1.1898x over previous
"""Trainium2 Bass kernel for the CSA (channel-spatial attention) module.

Reference computation (per batch b):
    q = Wq @ x[b]            # [64, N]
    k = Wk @ x[b]            # [64, N]
    E[n, m] = sum_c q[c, n] * k[c, m]          # [N, N]
    A = softmax(E, axis=m)
    v = Wv @ x_h[b]          # [128, N]
    out[c, n] = sum_m v[c, m] * A[n, m]
    result = gamma * out + x_h[b]

Sharding: 8 cores = 4 batches x 2 query-halves. Each core holds full K/V for
its batch and a 2048-wide query chunk (flash-style: the [N, N] attention
matrix is never materialized in HBM).

The kernel is ACT(exp)-roofline bound: 8.39M exps/core at 1 elem/cycle/lane
@1.2GHz = ~71us busy in 64 [128,1024] ACTIVATEs (PSUM's 8 banks cap the
pair size: e 2x2 + u 2 + s 1 + m 1). Everything else is organized to keep
that ACT stream dense and to minimize the prologue/tail around it:
- Host folds the tiny 1x1-conv projections: qk = (Wk^T Wq) @ x_half is
  precomputed on host (64x64 @ 64x2048 per core), as is gamma into Wv^T.
  The attention math itself (E/softmax/U = ~all FLOPs) runs on device.
- E^T computed in PAIRS of two m-tiles ([128,1024] f32 PSUM, 2 banks);
  one ACTIVATE exps each pair; U accumulates over m in PSUM; the softmax
  denominator folds pair sums on DVE so the ones-vector S matmul runs once
  per 4 m-tiles; S partials for the 2 last pairs go unfolded to shorten
  the group tail.
- Input DMA is bandwidth+latency limited (~2.3us engine startup): only
  2.3MB/core is loaded (no zero-pad rows, bf16 residual); E's stationary
  pad rows and qk's moving pad rows are zero-filled on-chip via DVE copies
  from a memset tile.
- PE warm-up matmuls at t=0 keep the HAM clock gate from idling cold;
  a 1-element dummy exp pulls the ~2.7us ACT_TABLE_LOAD off the critical
  path; vT projection blocks interleave into group 0's slack.
- out = U/S + x_h via reciprocal_approx_fast + gpsimd partition_broadcast
  (PE and ACT stay out of the epilogue).
- No max-subtraction: logits are N(0, 64); |E| < ~65 << 88 (exp overflow).
"""

import numpy as np

import concourse.bass as bass
import concourse.mybir as mybir
import concourse.tile as tile
from concourse import bacc
from concourse.bass_utils import run_bass_kernel_spmd

B = 4
CQK = 64
CV = 128
N = 4096
NQ = N // 2          # query columns per core
NG = 512             # n-group width (PSUM bank / U matmul free dim)
MT = 128             # m-tile height (PE contraction tile)
PW = 2 * NG          # E-pair width: 2 m-tiles side by side (2 PSUM banks f32)
N_GROUPS = NQ // NG  # 4
N_PAIRS_G = N // (2 * MT)   # 16 pairs per group
NPT = N_GROUPS * N_PAIRS_G  # 64 total pairs
N_WARM = 6           # PE warm-up matmuls (fill the DMA wait, prime HAM)
PIPE = 2             # E-pair pipeline depth

F32 = mybir.dt.float32
BF16 = mybir.dt.bfloat16


_last_results = None  # stashed BassKernelResults for test harnesses


def build_bass() -> bass.Bass:
    nc = bacc.Bacc()

    xb = nc.declare_dram_parameter("xb", [CQK, N], BF16, isOutput=False)
    xhb = nc.declare_dram_parameter("xhb", [CV, N], BF16, isOutput=False)
    qkh = nc.declare_dram_parameter("qkh", [CQK, NQ], BF16, isOutput=False)
    xh_res = nc.declare_dram_parameter("xh_res", [CV, NQ], BF16, isOutput=False)
    wvT = nc.declare_dram_parameter("wvT", [CV, CV], BF16, isOutput=False)
    o = nc.declare_dram_parameter("o", [CV, NQ], F32, isOutput=True)

    ts = bass.ts

    with tile.TileContext(nc) as tc:
        with (
            nc.allow_low_precision(reason="bf16 attention math, fp32 accum"),
            tc.tile_pool(name="const", bufs=1) as cpool,
            tc.tile_pool(name="pt", bufs=3) as ptpool,
            tc.tile_pool(name="red", bufs=2) as rpool,
            tc.tile_pool(name="ep", bufs=PIPE, space="PSUM") as epool,
            tc.tile_pool(name="up", bufs=2, space="PSUM") as upool,
            tc.tile_pool(name="sp", bufs=1, space="PSUM") as spool,
            tc.tile_pool(name="mp", bufs=1, space="PSUM") as mpool,
            tc.tile_pool(name="out", bufs=3) as opool,
        ):
            # ---- persistent SBUF tensors ----
            xb_sb = cpool.tile([MT, N], BF16)    # rows CQK.. zero-filled on-chip
            xhb_sb = cpool.tile([CV, N], BF16)
            xhres_sb = cpool.tile([CV, NQ], BF16)
            wvT_sb = cpool.tile([CV, CV], BF16)
            qk_sb = cpool.tile([MT, NQ], BF16)   # rows CQK.. zero-filled on-chip
            vT_sb = cpool.tile([CV, N], BF16)    # cols [mt*128,..) = v[:, chunk].T
            zwarm = cpool.tile([MT, NG], BF16)   # zeros: warm-up + pad source
            ones_m = cpool.tile([MT, 1], BF16)   # S-matmul stationary
            ones_stage = cpool.tile([MT, 1], F32)

            # ---- t=0: warm-up + table preload + DMA prologue ----
            nc.gpsimd.memset(zwarm[:], 0.0)

            # preload the exp table set while the DMAs run
            tl_sb = opool.tile([MT, 1], F32, tag="o", name="tl")
            nc.scalar.activation(tl_sb[:], zwarm[:, :1],
                                 mybir.ActivationFunctionType.Exp, bias=0.0)

            # warm the PE while the first DMAs are in flight (tiles spread
            # across two PSUM pools so the WAW chain doesn't serialize them)
            for w in range(N_WARM):
                wpool = mpool if w % 2 == 0 else upool
                wm = wpool.tile([CV, NG], F32,
                                tag="mpsum" if w % 2 == 0 else "u",
                                name=f"warm_{w}")
                nc.tensor.matmul(wm[:], zwarm[:, :MT], zwarm[:],
                                 start=True, stop=True)

            # sync queue: loop-critical tensors first, in consumption order.
            # Descriptors serialize at ~1-3us per 256KB on the shared DMA
            # engines, so the leading chunks are small: E's first pairs can
            # start ~4us before the bulk lands.
            nc.sync.dma_start(qk_sb[:CQK, :NG], qkh[:, :NG])
            nc.sync.dma_start(xb_sb[:CQK, ts(0, NG)], xb[:, ts(0, NG)])
            nc.sync.dma_start(wvT_sb[:], wvT[:])
            nc.sync.dma_start(xb_sb[:CQK, ts(1, NG)], xb[:, ts(1, NG)])
            nc.sync.dma_start(qk_sb[:CQK, NG:], qkh[:, NG:])
            for j in range(2, N // NG):
                nc.sync.dma_start(xb_sb[:CQK, ts(j, NG)], xb[:, ts(j, NG)])
            nc.sync.dma_start(xhres_sb[:], xh_res[:])
            # gpsimd queue: vT inputs, one chunk per vblk
            for j in range(N // NG):
                nc.gpsimd.dma_start(xhb_sb[:, ts(j, NG)], xhb[:, ts(j, NG)])
            nc.gpsimd.memset(ones_stage[:], 1.0)

            # on-chip zero padding (DVE; zwarm is the zero source):
            # qk rows CQK..127 (E's moving operand), then xb rows CQK..127
            # (E's stationary) copied from the freshly zeroed qk pad.
            for j in range(NQ // NG):
                nc.vector.tensor_copy(qk_sb[CQK:, ts(j, NG)], zwarm[:CQK, :])
            for j in range(2):
                nc.vector.tensor_copy(xb_sb[CQK:, ts(j, 2048)], qk_sb[CQK:, :])
            nc.vector.tensor_copy(ones_m[:], ones_stage[:])

            # ---- vT projection block j: vT[m, c] for m in [j*512,(j+1)*512) ----
            def emit_vblk(j):
                vt_ps = mpool.tile([CV, NG], F32, tag="mpsum", name=f"vtp_{j}")
                for u in range(4):
                    mt = j * 4 + u
                    nc.tensor.matmul(vt_ps[:, ts(u, MT)], xhb_sb[:, ts(mt, MT)],
                                     wvT_sb[:], start=True, stop=True)
                nc.vector.tensor_copy(vT_sb[:, ts(j, NG)], vt_ps[:])

            # ---- E-pair: two m-tiles' E^T for one n-group, f32 PSUM ----
            def emit_Epair(g, q):
                e2 = epool.tile([MT, PW], F32, tag="e", name=f"e_{g}_{q}")
                for u in range(2):
                    mt = q * 2 + u
                    nc.tensor.matmul(e2[:, ts(u, NG)], xb_sb[:, ts(mt, MT)],
                                     qk_sb[:, ts(g, NG)], start=True, stop=True)
                return e2

            def emit_epilogue(g, u_ps, s_ps, split=1):
                # out = U / S + x_h   (gamma pre-folded into wvT on the host)
                # split=2 pipelines the serial chain in column halves; used
                # for the last group where the chain is the kernel tail.
                w = NG // split
                for h in range(split):
                    sl = slice(h * w, (h + 1) * w)
                    r_sb = opool.tile([1, w], F32, tag="r", name=f"r_{g}_{h}")
                    nc.vector.reciprocal_approx_fast(out=r_sb[:],
                                                     in_=s_ps[:1, sl])
                    rb_sb = opool.tile([CV, w], F32, tag="rb",
                                       name=f"rb_{g}_{h}")
                    nc.gpsimd.partition_broadcast(rb_sb[:], r_sb[:])
                    o_sb = opool.tile([CV, w], F32, tag="o", name=f"o_{g}_{h}")
                    nc.vector.tensor_mul(o_sb[:], u_ps[:, sl], rb_sb[:])
                    nc.vector.tensor_add(o_sb[:], o_sb[:],
                                         xhres_sb[:, g * NG + h * w:
                                                   g * NG + (h + 1) * w])
                    nc.sync.dma_start(o[:, g * NG + h * w:
                                        g * NG + (h + 1) * w], o_sb[:])

            def emit_S(s_ps, first, last, ptq):
                nc.tensor.matmul(s_ps[:1, :], ones_m[:], ptq[:],
                                 start=first, stop=last)

            # ---- main flash loop over 64 pairs, software-pipelined ----
            e_tiles = {p: emit_Epair(p // N_PAIRS_G, p % N_PAIRS_G)
                       for p in range(PIPE)}
            for j in range(4):
                emit_vblk(j)
            u_ps = s_ps = None
            pending = None
            pending_s = []
            ptp_prev = None
            for p in range(NPT):
                g, q = divmod(p, N_PAIRS_G)
                if q == 0:
                    u_ps = upool.tile([CV, NG], F32, tag="u", name=f"u_{g}")
                    s_ps = spool.tile([1, NG], F32, tag="s", name=f"s_{g}")
                pt2 = ptpool.tile([MT, PW], BF16, tag="pt", name=f"pt_{g}_{q}")
                nc.scalar.activation(pt2[:], e_tiles.pop(p)[:],
                                     mybir.ActivationFunctionType.Exp,
                                     bias=0.0)
                if p + PIPE < NPT:
                    gn, qn = divmod(p + PIPE, N_PAIRS_G)
                    e_tiles[p + PIPE] = emit_Epair(gn, qn)
                # U[c, n] += vT_tile.T @ P^T  (both m-tiles of the pair)
                for u in range(2):
                    mt = q * 2 + u
                    nc.tensor.matmul(u_ps[:], vT_sb[:, ts(mt, MT)],
                                     pt2[:, ts(u, NG)],
                                     start=(q == 0 and u == 0),
                                     stop=(q == N_PAIRS_G - 1 and u == 1))
                if g == 0 and q < 4:
                    emit_vblk(q + 4)  # vblk j needed by U at pair 2j
                # pair reduction for the softmax denominator (DVE)
                ptp = rpool.tile([MT, NG], BF16, tag="ptp", name=f"pp_{g}_{q}")
                nc.vector.tensor_add(ptp[:], pt2[:, :NG], pt2[:, NG:])
                lastq = q == N_PAIRS_G - 1
                if q % 2 == 0:
                    ptp_prev = ptp
                elif lastq:
                    # final pairs go unfolded: shortens the group-tail chain
                    for args in pending_s:
                        emit_S(s_ps, *args)
                    pending_s = []
                    emit_S(s_ps, False, False, ptp_prev)
                    emit_S(s_ps, False, True, ptp)
                else:
                    # fold two pairs -> one S matmul (halves PE S work)
                    ptq = rpool.tile([MT, NG], BF16, tag="ptq",
                                     name=f"pq_{g}_{q}")
                    nc.vector.tensor_add(ptq[:], ptp_prev[:], ptp[:])
                    if q == 1:
                        pending_s.append((True, False, ptq))
                    elif q >= 3:
                        if pending_s:
                            for args in pending_s:
                                emit_S(s_ps, *args)
                            pending_s = []
                        emit_S(s_ps, False, False, ptq)
                if pending is not None and (q >= 1 or p == NPT - 1):
                    emit_epilogue(*pending)
                    pending = None
                if lastq:
                    pending = (g, u_ps, s_ps)
            emit_epilogue(*pending, split=2)

    nc.compile()
    return nc


def kernel(x, x_h, Wq, Wk, Wv, gamma):
    global _last_results
    import ml_dtypes
    bf16 = ml_dtypes.bfloat16

    x = np.ascontiguousarray(np.asarray(x, dtype=np.float32))
    x_h = np.ascontiguousarray(np.asarray(x_h, dtype=np.float32))
    Wq = np.asarray(Wq, dtype=np.float32)
    Wk = np.asarray(Wk, dtype=np.float32)
    Wv = np.asarray(Wv, dtype=np.float32)
    gval = float(np.asarray(gamma).reshape(-1)[0])

    nc = build_bass()

    # Fold the 1x1-conv projections on the host:
    #   qk = (Wk^T Wq) @ x_half  (query-key product, bf16)
    #   wvT = gamma * Wv^T       (value projection weights with gamma baked in)
    A = Wk.T @ Wq
    wvT_h = np.ascontiguousarray(Wv.T * gval).astype(bf16)
    x_bf = x.astype(bf16)
    xh_bf = x_h.astype(bf16)

    in_maps = []
    for core in range(8):
        b, h = core // 2, core % 2
        sl = slice(h * NQ, (h + 1) * NQ)
        in_maps.append({
            "xb": x_bf[b],
            "xhb": xh_bf[b],
            "qkh": np.ascontiguousarray((A @ x[b][:, sl]).astype(bf16)),
            "xh_res": np.ascontiguousarray(xh_bf[b][:, sl]),
            "wvT": wvT_h,
        })

    res = run_bass_kernel_spmd(nc, in_maps, list(range(8)))
    _last_results = res

    out = np.empty((B, CV, N), dtype=np.float32)
    for core in range(8):
        b, h = core // 2, core % 2
        out[b][:, h * NQ:(h + 1) * NQ] = res.results[core]["o"]
    return out
